# revision 1
# baseline (speedup 1.0000x reference)
"""Trainium2 Bass kernel for nn_Attention_Module (sparse_attention).

Computation per batch b (x_b: [C=256, T=4096] fp32):
    energy = x_b @ x_b^T                      # (256, 256), K=4096
    attn   = softmax(rowmax(energy) - energy) # == exp(mu - e)/Z, mu = rowmin
    out    = gamma * (attn @ x_b) + x_b

Strategy (8 cores, pure data-parallel, 4 batches/core):
  - All HBM I/O in fp16 (24 MB/core): x host-cast/staged into xt
    [P, KT, C] and xn [P, 2, T]; fp16 output host-upcast afterward.
  - Both matmuls in fp16 (1 col/cycle PE streams; f32r would be 1/4 rate).
  - +x residual folded into matmul2 via A'' = gamma*P^T + diag(Z);
    out = diag(1/Z) * (A''^T @ x).
  - Software-pipelined: batch b's matmul2 runs after batch b+1's matmul1.
"""

import numpy as np

B, C, T = 32, 256, 4096
NCORES = 8
NB = B // NCORES
P = 128
KT = T // P
KH = KT // 2
TC = T // 512
B0_BOUNDS = [0, 4, 8, 16, 24, 32]  # batch-0 graded chunk edges (k-tiles)

_CACHE = {}


def _build_nc():
    from contextlib import ExitStack

    import concourse.bacc as bacc
    import concourse.bass as bass
    import concourse.tile as tile
    from concourse import mybir

    f32 = mybir.dt.float32
    f16 = mybir.dt.float16
    ts = bass.ts

    nc = bacc.Bacc(
        "TRN2",
        target_bir_lowering=False,
        debug=False,
        enable_asserts=False,
        num_devices=NCORES,
    )

    xt_h = nc.dram_tensor("xt", [NB, P, KT, C], f16, kind="ExternalInput")
    xn_h = nc.dram_tensor("xn", [NB, P, 2, T], f16, kind="ExternalInput")
    aux_h = nc.dram_tensor("aux", [P, 132], f32, kind="ExternalInput")
    o_h = nc.dram_tensor("o", [NB, C, T], f16, kind="ExternalOutput")

    with tile.TileContext(nc) as tc:
        with ExitStack() as ctx:
            singles = ctx.enter_context(tc.tile_pool(name="singles", bufs=1))
            xq_pool = ctx.enter_context(tc.tile_pool(name="xq", bufs=1))
            xt_pool = ctx.enter_context(tc.tile_pool(name="xt", bufs=3))
            xn_pool = ctx.enter_context(tc.tile_pool(name="xn", bufs=4))
            out_pool = ctx.enter_context(tc.tile_pool(name="out", bufs=3))
            att_pool = ctx.enter_context(tc.tile_pool(name="att", bufs=3))
            small = ctx.enter_context(tc.tile_pool(name="small", bufs=4))
            psum_e = ctx.enter_context(
                tc.tile_pool(name="psum_e", bufs=2, space="PSUM")
            )
            psum_t = ctx.enter_context(
                tc.tile_pool(name="psum_t", bufs=2, space="PSUM")
            )
            psum_o = ctx.enter_context(
                tc.tile_pool(name="psum_o", bufs=4, space="PSUM")
            )

            xt_ap = xt_h.ap()
            xn_ap = xn_h.ap()
            o_ap = o_h.ap()

            aux = singles.tile([P, 132], f32)
            nc.scalar.dma_start(aux[:], aux_h.ap())
            gv = aux[:, 0:1]
            ident = aux[:, 4:132]
            identf = singles.tile([P, P], f16)
            nc.vector.tensor_copy(identf[:], ident)

            def issue_loads(b):
                if b == 0:
                    # graded chunks: first matmul starts after 128 KB lands
                    chunks = []
                    for ci in range(len(B0_BOUNDS) - 1):
                        lo, hi = B0_BOUNDS[ci], B0_BOUNDS[ci + 1]
                        t_ = xq_pool.tile(
                            [P, hi - lo, C], f16, tag=f"xq{ci}", name=f"xq{ci}"
                        )
                        nc.sync.dma_start(t_[:], xt_ap[b, :, lo:hi, :])
                        chunks.append((t_, lo, hi))
                else:
                    xta = xt_pool.tile([P, KH, C], f16, tag="xta", name="xta")
                    xtb = xt_pool.tile([P, KH, C], f16, tag="xtb", name="xtb")
                    nc.sync.dma_start(xta[:], xt_ap[b, :, :KH, :])
                    nc.sync.dma_start(xtb[:], xt_ap[b, :, KH:, :])
                    chunks = [(xta, 0, KH), (xtb, KH, KT)]
                xn = xn_pool.tile([P, 2, T], f16, tag="xn", name="xn")
                nc.sync.dma_start(xn[:], xn_ap[b])
                return chunks, xn

            def mm2_half(pb, pAt, prZ, pxn, m):
                ot = out_pool.tile([P, T], f16, tag="ot", name="ot")
                for t8 in range(TC):
                    po = psum_o.tile([P, 512], f32)
                    for k in range(2):
                        nc.tensor.matmul(
                            po[:],
                            lhsT=pAt[:, k, ts(m, P)],
                            rhs=pxn[:, k, ts(t8, 512)],
                            start=(k == 0),
                            stop=(k == 1),
                        )
                    if t8 % 2 == 0:
                        nc.vector.tensor_scalar_mul(
                            ot[:, ts(t8, 512)], po[:], prZ[:, m : m + 1]
                        )
                    else:
                        nc.scalar.mul(
                            ot[:, ts(t8, 512)], po[:], prZ[:, m : m + 1]
                        )
                nsplit = 4 if (pb == NB - 1 and m == 1) else 2
                for sh in range(nsplit):
                    nc.sync.dma_start(
                        o_ap[pb].rearrange("(m p) t -> p m t", p=P)[
                            :, m, ts(sh, T // nsplit)
                        ],
                        ot[:, ts(sh, T // nsplit)],
                    )
                return ot

            tiles = {0: issue_loads(0)}
            pending = None

            for b in range(NB):
                xt, xn = tiles.pop(b)
                if b + 1 < NB:
                    tiles[b + 1] = issue_loads(b + 1)

                At = att_pool.tile([P, 2, C], f16)
                Zs = small.tile([P, 2], f32)
                Zb = small.tile([P, 2], f16)
                rZ = small.tile([P, 2], f32)
                prev = pending
                pending = (b, At, rZ, xn)
                deferred = []

                for m in range(2):
                    pe = psum_e.tile([P, C], f32)
                    ci = 0
                    for k in range(KT):
                        while k >= xt[ci][2]:
                            ci += 1
                        src_t, lo, _ = xt[ci]
                        kk = k - lo
                        nc.tensor.matmul(
                            pe[:],
                            lhsT=src_t[:, kk, ts(m, P)],
                            rhs=src_t[:, kk, :],
                            start=(k == 0),
                            stop=(k == KT - 1),
                        )
                    mu = small.tile([P, 1], f32)
                    nc.vector.tensor_reduce(
                        mu[:], pe[:], axis=mybir.AxisListType.X,
                        op=mybir.AluOpType.min,
                    )
                    Pm = small.tile([P, C], f16, tag=f"Pm{m}")
                    nc.scalar.activation(
                        Pm[:],
                        pe[:],
                        mybir.ActivationFunctionType.Exp,
                        bias=mu[:],
                        scale=-1.0,
                        accum_out=Zs[:, m : m + 1],
                    )
                    nc.vector.tensor_copy(Zb[:, m : m + 1], Zs[:, m : m + 1])
                    nc.vector.reciprocal(rZ[:, m : m + 1], Zb[:, m : m + 1])

                    def build(m, Pm, idx):
                        for k in range(2):
                            pt = psum_t.tile([P, P], f16)
                            nc.tensor.transpose(pt[:], Pm[:, ts(k, P)], idx[:])
                            nc.scalar.mul(At[:, k, ts(m, P)], pt[:], gv)
                        dg = small.tile([P, P], f16, tag="diag")
                        nc.vector.tensor_scalar_mul(
                            dg[:], ident, Zs[:, m : m + 1]
                        )
                        nc.vector.tensor_add(
                            At[:, m, ts(m, P)], At[:, m, ts(m, P)], dg[:]
                        )

                    # previous batch's matmul2 half m hides this half's
                    # softmax latency on the PE
                    if prev is not None:
                        ot_prev = mm2_half(prev[0], prev[1], prev[2], prev[3], m)
                        # pin the A'' transposes AFTER this matmul2 half:
                        # rebuild their moving identity operand with a
                        # no-op that reads the half's evacuated output, so
                        # the scheduler cannot hoist the transposes into
                        # an exp-wait stall right behind matmul1
                        idx = small.tile([P, P], f16, tag="idx")
                        nc.vector.scalar_tensor_tensor(
                            idx[:],
                            ot_prev[:, 3 * 512 : 3 * 512 + P],
                            0.0,
                            identf[:],
                            op0=mybir.AluOpType.mult,
                            op1=mybir.AluOpType.add,
                        )
                        build(m, Pm, idx)
                    else:
                        # batch 0: defer builds past both softmax halves so
                        # the ACT queue runs [exp m0, exp m1] back-to-back
                        # and neither transpose stalls the PE on the chain
                        deferred.append((m, Pm))

                if prev is None:
                    for m_, Pm_ in deferred:
                        build(m_, Pm_, identf)

                if b == NB - 1:
                    mm2_half(b, At, rZ, xn, 0)
                    mm2_half(b, At, rZ, xn, 1)

    nc.compile()
    return nc


def _get_nc():
    if "nc" not in _CACHE:
        _CACHE["nc"] = _build_nc()
    return _CACHE["nc"]


def _make_aux(gamma_val):
    aux = np.zeros((P, 132), dtype=np.float32)
    aux[:, 0] = gamma_val
    aux[:, 1] = 1.0 / gamma_val if gamma_val != 0 else 0.0
    aux[:, 4:132] = np.eye(P, dtype=np.float32)
    return aux


def kernel(x, gamma, _trace=False):
    import concourse.bass_utils as bass_utils

    x = np.ascontiguousarray(np.asarray(x, dtype=np.float32))
    gamma = np.asarray(gamma, dtype=np.float32).reshape(-1)

    nc = _get_nc()

    aux = _make_aux(gamma[0])
    x16 = x.astype(np.float16)
    in_maps = []
    for d in range(NCORES):
        xs = x16[d * NB : (d + 1) * NB]
        xt = np.ascontiguousarray(
            xs.transpose(0, 2, 1).reshape(NB, KT, P, C).transpose(0, 2, 1, 3)
        )
        xn = np.ascontiguousarray(
            xs.reshape(NB, 2, P, T).transpose(0, 2, 1, 3)
        )
        in_maps.append({"xt": xt, "xn": xn, "aux": aux})

    res = bass_utils.run_bass_kernel_spmd(
        nc, in_maps, core_ids=list(range(NCORES)), trace=_trace
    )
    out = np.concatenate([r["o"] for r in res.results], axis=0).astype(
        np.float32
    )
    if _trace:
        _CACHE["last_results"] = res
    return out



# revision 3
# speedup vs baseline: 1.0071x; 1.0071x over previous
"""Trainium2 Bass kernel for nn_Attention_Module (sparse_attention).

Computation per batch b (x_b: [C=256, T=4096] fp32):
    energy = x_b @ x_b^T                      # (256, 256), K=4096
    attn   = softmax(rowmax(energy) - energy) # == exp(mu - e)/Z, mu = rowmin
    out    = gamma * (attn @ x_b) + x_b

Strategy (8 cores, pure data-parallel, 4 batches/core):
  - All HBM I/O in fp16 (24 MB/core): x host-cast/staged into xt
    [P, KT, C] and xn [P, 2, T]; fp16 output host-upcast afterward.
  - Both matmuls in fp16 (1 col/cycle PE streams; f32r would be 1/4 rate).
  - +x residual folded into matmul2 via A'' = gamma*P^T + diag(Z);
    out = diag(1/Z) * (A''^T @ x).
  - Software-pipelined: batch b's matmul2 runs after batch b+1's matmul1.
"""

import numpy as np

B, C, T = 32, 256, 4096
NCORES = 8
NB = B // NCORES
P = 128
KT = T // P
KH = KT // 2
TC = T // 512
B0_BOUNDS = [0, 4, 8, 16, 24, 32]  # batch-0 graded chunk edges (k-tiles)

_CACHE = {}


def _make_fast_tile_context(tile):
    """TileContext with a cheaper kernel exit: keep the final DMA drain but
    replace the two full all-engine barriers (per-engine InstDrain + double
    butterfly, ~7us) with a single sem-only butterfly before the semaphore
    clear."""
    from concourse.vector_clock import ScopedClock

    class FastExitTileContext(tile.TileContext):
        def _drain_and_barrier(self, tick_clock, wait_clock):
            drain_inst = self.nc.sync.drain()
            wait_clock.add_sem_waits(
                drain_inst.ins, ScopedClock({None: tick_clock.global_clock})
            )
            self.nc.all_engine_barrier(sem_only=True)
            popped = self.nc._tile_sem_poison_stack.pop()
            assert popped is self._sem_poison
            self.nc.clear_and_free_semaphores(
                list(self.sems.allocated().values())
            )

    return FastExitTileContext


def _build_nc():
    from contextlib import ExitStack

    import concourse.bacc as bacc
    import concourse.bass as bass
    import concourse.tile as tile
    from concourse import mybir

    f32 = mybir.dt.float32
    f16 = mybir.dt.float16
    ts = bass.ts

    nc = bacc.Bacc(
        "TRN2",
        target_bir_lowering=False,
        debug=False,
        enable_asserts=False,
        num_devices=NCORES,
    )

    xt_h = nc.dram_tensor("xt", [NB, P, KT, C], f16, kind="ExternalInput")
    xn_h = nc.dram_tensor("xn", [NB, P, 2, T], f16, kind="ExternalInput")
    aux_h = nc.dram_tensor("aux", [P, 132], f32, kind="ExternalInput")
    o_h = nc.dram_tensor("o", [NB, C, T], f16, kind="ExternalOutput")

    FastExitTileContext = _make_fast_tile_context(tile)
    with FastExitTileContext(nc) as tc:
        with ExitStack() as ctx:
            singles = ctx.enter_context(tc.tile_pool(name="singles", bufs=1))
            xq_pool = ctx.enter_context(tc.tile_pool(name="xq", bufs=1))
            xt_pool = ctx.enter_context(tc.tile_pool(name="xt", bufs=3))
            xn_pool = ctx.enter_context(tc.tile_pool(name="xn", bufs=4))
            out_pool = ctx.enter_context(tc.tile_pool(name="out", bufs=3))
            att_pool = ctx.enter_context(tc.tile_pool(name="att", bufs=3))
            small = ctx.enter_context(tc.tile_pool(name="small", bufs=4))
            psum_e = ctx.enter_context(
                tc.tile_pool(name="psum_e", bufs=2, space="PSUM")
            )
            psum_t = ctx.enter_context(
                tc.tile_pool(name="psum_t", bufs=2, space="PSUM")
            )
            psum_o = ctx.enter_context(
                tc.tile_pool(name="psum_o", bufs=4, space="PSUM")
            )

            xt_ap = xt_h.ap()
            xn_ap = xn_h.ap()
            o_ap = o_h.ap()

            aux = singles.tile([P, 132], f32)
            nc.scalar.dma_start(aux[:], aux_h.ap())
            gv = aux[:, 0:1]
            ident = aux[:, 4:132]
            identf = singles.tile([P, P], f16)
            nc.vector.tensor_copy(identf[:], ident)

            def issue_loads(b):
                if b == 0:
                    # graded chunks: first matmul starts after 128 KB lands
                    chunks = []
                    for ci in range(len(B0_BOUNDS) - 1):
                        lo, hi = B0_BOUNDS[ci], B0_BOUNDS[ci + 1]
                        t_ = xq_pool.tile(
                            [P, hi - lo, C], f16, tag=f"xq{ci}", name=f"xq{ci}"
                        )
                        nc.sync.dma_start(t_[:], xt_ap[b, :, lo:hi, :])
                        chunks.append((t_, lo, hi))
                else:
                    xta = xt_pool.tile([P, KH, C], f16, tag="xta", name="xta")
                    xtb = xt_pool.tile([P, KH, C], f16, tag="xtb", name="xtb")
                    nc.sync.dma_start(xta[:], xt_ap[b, :, :KH, :])
                    nc.sync.dma_start(xtb[:], xt_ap[b, :, KH:, :])
                    chunks = [(xta, 0, KH), (xtb, KH, KT)]
                xn = xn_pool.tile([P, 2, T], f16, tag="xn", name="xn")
                nc.sync.dma_start(xn[:], xn_ap[b])
                return chunks, xn

            def mm2_half(pb, pAt, prZ, pxn, m):
                ot = out_pool.tile([P, T], f16, tag="ot", name="ot")
                for t8 in range(TC):
                    po = psum_o.tile([P, 512], f32)
                    for k in range(2):
                        nc.tensor.matmul(
                            po[:],
                            lhsT=pAt[:, k, ts(m, P)],
                            rhs=pxn[:, k, ts(t8, 512)],
                            start=(k == 0),
                            stop=(k == 1),
                        )
                    if t8 % 2 == 0:
                        nc.vector.tensor_scalar_mul(
                            ot[:, ts(t8, 512)], po[:], prZ[:, m : m + 1]
                        )
                    else:
                        nc.scalar.mul(
                            ot[:, ts(t8, 512)], po[:], prZ[:, m : m + 1]
                        )
                nsplit = 4 if (pb == NB - 1 and m == 1) else 2
                for sh in range(nsplit):
                    nc.sync.dma_start(
                        o_ap[pb].rearrange("(m p) t -> p m t", p=P)[
                            :, m, ts(sh, T // nsplit)
                        ],
                        ot[:, ts(sh, T // nsplit)],
                    )
                return ot

            tiles = {0: issue_loads(0)}
            pending = None

            for b in range(NB):
                xt, xn = tiles.pop(b)
                if b + 1 < NB:
                    tiles[b + 1] = issue_loads(b + 1)

                At = att_pool.tile([P, 2, C], f16)
                Zs = small.tile([P, 2], f32)
                Zb = small.tile([P, 2], f16)
                rZ = small.tile([P, 2], f32)
                prev = pending
                pending = (b, At, rZ, xn)
                deferred = []

                for m in range(2):
                    pe = psum_e.tile([P, C], f32)
                    ci = 0
                    for k in range(KT):
                        while k >= xt[ci][2]:
                            ci += 1
                        src_t, lo, _ = xt[ci]
                        kk = k - lo
                        nc.tensor.matmul(
                            pe[:],
                            lhsT=src_t[:, kk, ts(m, P)],
                            rhs=src_t[:, kk, :],
                            start=(k == 0),
                            stop=(k == KT - 1),
                        )
                    mu = small.tile([P, 1], f32)
                    nc.vector.tensor_reduce(
                        mu[:], pe[:], axis=mybir.AxisListType.X,
                        op=mybir.AluOpType.min,
                    )
                    Pm = small.tile([P, C], f16, tag=f"Pm{m}")
                    nc.scalar.activation(
                        Pm[:],
                        pe[:],
                        mybir.ActivationFunctionType.Exp,
                        bias=mu[:],
                        scale=-1.0,
                        accum_out=Zs[:, m : m + 1],
                    )
                    nc.vector.tensor_copy(Zb[:, m : m + 1], Zs[:, m : m + 1])
                    nc.vector.reciprocal(rZ[:, m : m + 1], Zb[:, m : m + 1])

                    def build(m, Pm, idx):
                        for k in range(2):
                            pt = psum_t.tile([P, P], f16)
                            nc.tensor.transpose(pt[:], Pm[:, ts(k, P)], idx[:])
                            nc.scalar.mul(At[:, k, ts(m, P)], pt[:], gv)
                        dg = small.tile([P, P], f16, tag="diag")
                        nc.vector.tensor_scalar_mul(
                            dg[:], ident, Zs[:, m : m + 1]
                        )
                        nc.vector.tensor_add(
                            At[:, m, ts(m, P)], At[:, m, ts(m, P)], dg[:]
                        )

                    # previous batch's matmul2 half m hides this half's
                    # softmax latency on the PE
                    if prev is not None:
                        ot_prev = mm2_half(prev[0], prev[1], prev[2], prev[3], m)
                        # pin the A'' transposes AFTER this matmul2 half:
                        # rebuild their moving identity operand with a
                        # no-op that reads the half's evacuated output, so
                        # the scheduler cannot hoist the transposes into
                        # an exp-wait stall right behind matmul1
                        idx = small.tile([P, P], f16, tag="idx")
                        nc.vector.scalar_tensor_tensor(
                            idx[:],
                            ot_prev[:, 3 * 512 : 3 * 512 + P],
                            0.0,
                            identf[:],
                            op0=mybir.AluOpType.mult,
                            op1=mybir.AluOpType.add,
                        )
                        build(m, Pm, idx)
                    else:
                        # batch 0: defer builds past both softmax halves so
                        # the ACT queue runs [exp m0, exp m1] back-to-back
                        # and neither transpose stalls the PE on the chain
                        deferred.append((m, Pm))

                if prev is None:
                    for m_, Pm_ in deferred:
                        build(m_, Pm_, identf)

                if b == NB - 1:
                    mm2_half(b, At, rZ, xn, 0)
                    mm2_half(b, At, rZ, xn, 1)

    nc.compile()
    return nc


def _get_nc():
    if "nc" not in _CACHE:
        _CACHE["nc"] = _build_nc()
    return _CACHE["nc"]


def _make_aux(gamma_val):
    aux = np.zeros((P, 132), dtype=np.float32)
    aux[:, 0] = gamma_val
    aux[:, 1] = 1.0 / gamma_val if gamma_val != 0 else 0.0
    aux[:, 4:132] = np.eye(P, dtype=np.float32)
    return aux


def kernel(x, gamma, _trace=False):
    import concourse.bass_utils as bass_utils

    x = np.ascontiguousarray(np.asarray(x, dtype=np.float32))
    gamma = np.asarray(gamma, dtype=np.float32).reshape(-1)

    nc = _get_nc()

    aux = _make_aux(gamma[0])
    x16 = x.astype(np.float16)
    in_maps = []
    for d in range(NCORES):
        xs = x16[d * NB : (d + 1) * NB]
        xt = np.ascontiguousarray(
            xs.transpose(0, 2, 1).reshape(NB, KT, P, C).transpose(0, 2, 1, 3)
        )
        xn = np.ascontiguousarray(
            xs.reshape(NB, 2, P, T).transpose(0, 2, 1, 3)
        )
        in_maps.append({"xt": xt, "xn": xn, "aux": aux})

    res = bass_utils.run_bass_kernel_spmd(
        nc, in_maps, core_ids=list(range(NCORES)), trace=_trace
    )
    out = np.concatenate([r["o"] for r in res.results], axis=0).astype(
        np.float32
    )
    if _trace:
        _CACHE["last_results"] = res
    return out



# revision 4
# speedup vs baseline: 1.0442x; 1.0368x over previous
"""Trainium2 Bass kernel for nn_Attention_Module (sparse_attention).

Computation per batch b (x_b: [C=256, T=4096] fp32):
    energy = x_b @ x_b^T                      # (256, 256), K=4096
    attn   = softmax(rowmax(energy) - energy) # == exp(mu - e)/Z, mu = rowmin
    out    = gamma * (attn @ x_b) + x_b

Strategy (8 cores, pure data-parallel, 4 batches/core):
  - mm1 (energy) in fp16 from xt [P, KT, C] (x^T tiles, 8 MB/core).
  - mm2 (attn @ x) in fp8e4 with DoubleRow (2 fp8 MACs/PE cell): stationary
    At8 = P^T (exp matrix, values in (0,1]), moving xn8 = fp8(x) staged from
    host (4 MB/core instead of 8 MB fp16 -> 20 MB total HBM traffic).
  - gamma/Z folded into the PSUM evacuation scale (gamma/Z per row).
  - Device returns U = gamma * attn @ x (fp16); the +x residual is merged on
    host in fp32 during unshard (more precise than a device fp16 add).
  - Software-pipelined: batch b's matmul2 runs after batch b+1's matmul1.
  - Cheap kernel exit: single sem-only butterfly instead of two full
    all-engine barriers.
"""

import numpy as np

B, C, T = 32, 256, 4096
NCORES = 8
NB = B // NCORES
P = 128
KT = T // P
KH = KT // 2
TC = T // 512
B0_BOUNDS = [0, 4, 8, 16, 24, 32]  # batch-0 graded chunk edges (k-tiles)

_CACHE = {}


def _make_fast_tile_context(tile):
    """TileContext with a cheaper kernel exit: keep the final DMA drain but
    replace the two full all-engine barriers (per-engine InstDrain + double
    butterfly) with a single sem-only butterfly before the semaphore
    clear."""
    from concourse.vector_clock import ScopedClock

    class FastExitTileContext(tile.TileContext):
        def _drain_and_barrier(self, tick_clock, wait_clock):
            drain_inst = self.nc.sync.drain()
            wait_clock.add_sem_waits(
                drain_inst.ins, ScopedClock({None: tick_clock.global_clock})
            )
            self.nc.all_engine_barrier(sem_only=True)
            popped = self.nc._tile_sem_poison_stack.pop()
            assert popped is self._sem_poison
            self.nc.clear_and_free_semaphores(
                list(self.sems.allocated().values())
            )

    return FastExitTileContext


def _build_nc():
    from contextlib import ExitStack

    import concourse.bacc as bacc
    import concourse.bass as bass
    import concourse.tile as tile
    from concourse import mybir

    f32 = mybir.dt.float32
    f16 = mybir.dt.float16
    f8 = mybir.dt.float8e4
    DR = mybir.MatmulPerfMode.DoubleRow
    ts = bass.ts

    nc = bacc.Bacc(
        "TRN2",
        target_bir_lowering=False,
        debug=False,
        enable_asserts=False,
        num_devices=NCORES,
    )

    xt_h = nc.dram_tensor("xt", [NB, P, KT, C], f16, kind="ExternalInput")
    xn_h = nc.dram_tensor("xn", [NB, P, 2, T], f8, kind="ExternalInput")
    aux_h = nc.dram_tensor("aux", [P, 132], f32, kind="ExternalInput")
    o_h = nc.dram_tensor("o", [NB, C, T], f16, kind="ExternalOutput")

    FastExitTileContext = _make_fast_tile_context(tile)
    with FastExitTileContext(nc) as tc:
        with ExitStack() as ctx:
            singles = ctx.enter_context(tc.tile_pool(name="singles", bufs=1))
            xq_pool = ctx.enter_context(tc.tile_pool(name="xq", bufs=1))
            xt_pool = ctx.enter_context(tc.tile_pool(name="xt", bufs=3))
            xn_pool = ctx.enter_context(tc.tile_pool(name="xn", bufs=4))
            out_pool = ctx.enter_context(tc.tile_pool(name="out", bufs=3))
            att_pool = ctx.enter_context(tc.tile_pool(name="att", bufs=3))
            small = ctx.enter_context(tc.tile_pool(name="small", bufs=4))
            psum_e = ctx.enter_context(
                tc.tile_pool(name="psum_e", bufs=2, space="PSUM")
            )
            psum_t = ctx.enter_context(
                tc.tile_pool(name="psum_t", bufs=2, space="PSUM")
            )
            psum_o = ctx.enter_context(
                tc.tile_pool(name="psum_o", bufs=4, space="PSUM")
            )

            xt_ap = xt_h.ap()
            xn_ap = xn_h.ap()
            o_ap = o_h.ap()

            aux = singles.tile([P, 132], f32)
            nc.scalar.dma_start(aux[:], aux_h.ap())
            rgv = aux[:, 1:2]   # 1/gamma
            onev = aux[:, 2:3]  # 1.0
            ident = aux[:, 4:132]
            identf = singles.tile([P, P], f16)
            nc.vector.tensor_copy(identf[:], ident)

            def issue_loads(b):
                if b == 0:
                    # graded chunks: first matmul starts after first chunk
                    chunks = []
                    for ci in range(len(B0_BOUNDS) - 1):
                        lo, hi = B0_BOUNDS[ci], B0_BOUNDS[ci + 1]
                        t_ = xq_pool.tile(
                            [P, hi - lo, C], f16, tag=f"xq{ci}", name=f"xq{ci}"
                        )
                        nc.sync.dma_start(t_[:], xt_ap[b, :, lo:hi, :])
                        chunks.append((t_, lo, hi))
                else:
                    xta = xt_pool.tile([P, KH, C], f16, tag="xta", name="xta")
                    xtb = xt_pool.tile([P, KH, C], f16, tag="xtb", name="xtb")
                    nc.sync.dma_start(xta[:], xt_ap[b, :, :KH, :])
                    nc.sync.dma_start(xtb[:], xt_ap[b, :, KH:, :])
                    chunks = [(xta, 0, KH), (xtb, KH, KT)]
                xn = xn_pool.tile([P, 2, T], f8, tag="xn", name="xn")
                nc.sync.dma_start(xn[:], xn_ap[b])
                return chunks, xn

            def mm2_half(pb, pAt, prZ, pxn, m):
                ot = out_pool.tile([P, T], f16, tag="ot", name="ot")
                for t8 in range(TC):
                    po = psum_o.tile([P, 512], f32)
                    nc.tensor.matmul(
                        po[:],
                        lhsT=pAt[:, :, ts(m, P)],
                        rhs=pxn[:, :, ts(t8, 512)],
                        start=True,
                        stop=True,
                        perf_mode=DR,
                    )
                    if t8 % 2 == 0:
                        nc.vector.tensor_scalar_mul(
                            ot[:, ts(t8, 512)], po[:], prZ[:, m : m + 1]
                        )
                    else:
                        nc.scalar.mul(
                            ot[:, ts(t8, 512)], po[:], prZ[:, m : m + 1]
                        )
                nsplit = 4 if (pb == NB - 1 and m == 1) else 2
                for sh in range(nsplit):
                    nc.sync.dma_start(
                        o_ap[pb].rearrange("(m p) t -> p m t", p=P)[
                            :, m, ts(sh, T // nsplit)
                        ],
                        ot[:, ts(sh, T // nsplit)],
                    )
                return ot

            tiles = {0: issue_loads(0)}
            pending = None

            for b in range(NB):
                xt, xn = tiles.pop(b)
                if b + 1 < NB:
                    tiles[b + 1] = issue_loads(b + 1)

                At = att_pool.tile([P, 2, C], f8)
                Zs = small.tile([P, 2], f32)
                Zb = small.tile([P, 2], f16)
                rZ = small.tile([P, 2], f32)
                prev = pending
                pending = (b, At, rZ, xn)
                deferred = []

                for m in range(2):
                    pe = psum_e.tile([P, C], f32)
                    ci = 0
                    for k in range(KT):
                        while k >= xt[ci][2]:
                            ci += 1
                        src_t, lo, _ = xt[ci]
                        kk = k - lo
                        nc.tensor.matmul(
                            pe[:],
                            lhsT=src_t[:, kk, ts(m, P)],
                            rhs=src_t[:, kk, :],
                            start=(k == 0),
                            stop=(k == KT - 1),
                        )
                    mu = small.tile([P, 1], f32)
                    nc.vector.tensor_reduce(
                        mu[:], pe[:], axis=mybir.AxisListType.X,
                        op=mybir.AluOpType.min,
                    )
                    Pm = small.tile([P, C], f16, tag=f"Pm{m}")
                    nc.scalar.activation(
                        Pm[:],
                        pe[:],
                        mybir.ActivationFunctionType.Exp,
                        bias=mu[:],
                        scale=-1.0,
                        accum_out=Zs[:, m : m + 1],
                    )
                    # Zb = Z/gamma (f16), rZ = gamma/Z (f32)
                    nc.vector.tensor_scalar_mul(
                        Zb[:, m : m + 1], Zs[:, m : m + 1], rgv
                    )
                    nc.vector.reciprocal(rZ[:, m : m + 1], Zb[:, m : m + 1])

                    def build(m, Pm, idx):
                        for k in range(2):
                            pt = psum_t.tile([P, P], f16)
                            nc.tensor.transpose(pt[:], Pm[:, ts(k, P)], idx[:])
                            nc.scalar.mul(At[:, k, ts(m, P)], pt[:], onev)

                    # previous batch's matmul2 half m hides this half's
                    # softmax latency on the PE
                    if prev is not None:
                        ot_prev = mm2_half(prev[0], prev[1], prev[2], prev[3], m)
                        # pin the At transposes AFTER this matmul2 half:
                        # rebuild their moving identity operand with a
                        # no-op that reads the half's evacuated output, so
                        # the scheduler cannot hoist the transposes into
                        # an exp-wait stall right behind matmul1
                        idx = small.tile([P, P], f16, tag="idx")
                        nc.vector.scalar_tensor_tensor(
                            idx[:],
                            ot_prev[:, 3 * 512 : 3 * 512 + P],
                            0.0,
                            identf[:],
                            op0=mybir.AluOpType.mult,
                            op1=mybir.AluOpType.add,
                        )
                        build(m, Pm, idx)
                    else:
                        # batch 0: defer builds past both softmax halves so
                        # the ACT queue runs [exp m0, exp m1] back-to-back
                        # and neither transpose stalls the PE on the chain
                        deferred.append((m, Pm))

                if prev is None:
                    for m_, Pm_ in deferred:
                        build(m_, Pm_, identf)

                if b == NB - 1:
                    mm2_half(b, At, rZ, xn, 0)
                    mm2_half(b, At, rZ, xn, 1)

    nc.compile()
    return nc


def _get_nc():
    if "nc" not in _CACHE:
        _CACHE["nc"] = _build_nc()
    return _CACHE["nc"]


def _make_aux(gamma_val):
    aux = np.zeros((P, 132), dtype=np.float32)
    aux[:, 0] = gamma_val
    aux[:, 1] = 1.0 / gamma_val if gamma_val != 0 else 0.0
    aux[:, 2] = 1.0
    aux[:, 4:132] = np.eye(P, dtype=np.float32)
    return aux


def kernel(x, gamma, _trace=False):
    import ml_dtypes

    import concourse.bass_utils as bass_utils

    x = np.ascontiguousarray(np.asarray(x, dtype=np.float32))
    gamma = np.asarray(gamma, dtype=np.float32).reshape(-1)

    nc = _get_nc()

    aux = _make_aux(gamma[0])
    x16 = x.astype(np.float16)
    in_maps = []
    for d in range(NCORES):
        xs16 = x16[d * NB : (d + 1) * NB]
        xt = np.ascontiguousarray(
            xs16.transpose(0, 2, 1).reshape(NB, KT, P, C).transpose(0, 2, 1, 3)
        )
        xs = x[d * NB : (d + 1) * NB]
        xn = np.ascontiguousarray(
            xs.reshape(NB, 2, P, T).transpose(0, 2, 1, 3)
        ).astype(ml_dtypes.float8_e4m3)
        in_maps.append({"xt": xt, "xn": xn, "aux": aux})

    res = bass_utils.run_bass_kernel_spmd(
        nc, in_maps, core_ids=list(range(NCORES)), trace=_trace
    )
    # device returns U = gamma * attn @ x (fp16); residual +x merged here
    out = np.concatenate([r["o"] for r in res.results], axis=0).astype(
        np.float32
    )
    out += x
    if _trace:
        _CACHE["last_results"] = res
    return out


# revision 8
# speedup vs baseline: 1.0480x; 1.0037x over previous
"""Trainium2 Bass kernel for nn_Attention_Module (sparse_attention).

Computation per batch b (x_b: [C=256, T=4096] fp32):
    energy = x_b @ x_b^T                      # (256, 256), K=4096
    attn   = softmax(rowmax(energy) - energy) # == exp(mu - e)/Z, mu = rowmin
    out    = gamma * (attn @ x_b) + x_b

Strategy (8 cores, pure data-parallel, 4 batches/core):
  - mm1 (energy) in fp16 from xt [P, KT, C] (x^T tiles, 8 MB/core).
  - mm2 (attn @ x) in fp8e4 with DoubleRow (2 fp8 MACs/PE cell): stationary
    At8 = P^T (exp matrix, values in (0,1]), moving xn8 = fp8(x) staged from
    host (4 MB/core instead of 8 MB fp16 -> 20 MB total HBM traffic).
  - gamma/Z folded into the PSUM evacuation scale (gamma/Z per row).
  - Device returns U = gamma * attn @ x (fp16); the +x residual is merged on
    host in fp32 during unshard (more precise than a device fp16 add).
  - Software-pipelined: batch b's matmul2 runs after batch b+1's matmul1.
  - Cheap kernel exit: single sem-only butterfly instead of two full
    all-engine barriers.
"""

import numpy as np

B, C, T = 32, 256, 4096
NCORES = 8
NB = B // NCORES
P = 128
KT = T // P
KH = KT // 2
TC = T // 512
B0_BOUNDS = [0, 4, 8, 16, 24, 32]  # batch-0 graded chunk edges (k-tiles)

_CACHE = {}


def _make_fast_tile_context(tile):
    """TileContext with a cheaper kernel exit: keep the final DMA drain but
    replace the two full all-engine barriers (per-engine InstDrain + double
    butterfly) with a single sem-only butterfly before the semaphore
    clear."""
    from concourse.vector_clock import ScopedClock

    class FastExitTileContext(tile.TileContext):
        def _drain_and_barrier(self, tick_clock, wait_clock):
            drain_inst = self.nc.sync.drain()
            wait_clock.add_sem_waits(
                drain_inst.ins, ScopedClock({None: tick_clock.global_clock})
            )
            self.nc.all_engine_barrier(sem_only=True)
            popped = self.nc._tile_sem_poison_stack.pop()
            assert popped is self._sem_poison
            self.nc.clear_and_free_semaphores(
                list(self.sems.allocated().values())
            )

    return FastExitTileContext


def _build_nc():
    from contextlib import ExitStack

    import concourse.bacc as bacc
    import concourse.bass as bass
    import concourse.tile as tile
    from concourse import mybir

    f32 = mybir.dt.float32
    f16 = mybir.dt.float16
    f8 = mybir.dt.float8e4
    DR = mybir.MatmulPerfMode.DoubleRow
    ts = bass.ts

    nc = bacc.Bacc(
        "TRN2",
        target_bir_lowering=False,
        debug=False,
        enable_asserts=False,
        num_devices=NCORES,
    )

    xt_h = nc.dram_tensor("xt", [NB, P, KT, C], f16, kind="ExternalInput")
    xn_h = nc.dram_tensor("xn", [NB, P, 2, T], f8, kind="ExternalInput")
    aux_h = nc.dram_tensor("aux", [P, 132], f32, kind="ExternalInput")
    o_h = nc.dram_tensor("o", [NB, C, T], f16, kind="ExternalOutput")

    FastExitTileContext = _make_fast_tile_context(tile)
    with FastExitTileContext(nc) as tc:
        with ExitStack() as ctx:
            singles = ctx.enter_context(tc.tile_pool(name="singles", bufs=1))
            xq_pool = ctx.enter_context(tc.tile_pool(name="xq", bufs=1))
            xt_pool = ctx.enter_context(tc.tile_pool(name="xt", bufs=3))
            xn_pool = ctx.enter_context(tc.tile_pool(name="xn", bufs=4))
            out_pool = ctx.enter_context(tc.tile_pool(name="out", bufs=3))
            att_pool = ctx.enter_context(tc.tile_pool(name="att", bufs=3))
            small = ctx.enter_context(tc.tile_pool(name="small", bufs=4))
            psum_e = ctx.enter_context(
                tc.tile_pool(name="psum_e", bufs=2, space="PSUM")
            )
            psum_t = ctx.enter_context(
                tc.tile_pool(name="psum_t", bufs=2, space="PSUM")
            )
            psum_o = ctx.enter_context(
                tc.tile_pool(name="psum_o", bufs=4, space="PSUM")
            )

            xt_ap = xt_h.ap()
            xn_ap = xn_h.ap()
            o_ap = o_h.ap()

            aux = singles.tile([P, 132], f32)
            nc.scalar.dma_start(aux[:], aux_h.ap())
            rgv = aux[:, 1:2]   # 1/gamma
            onev = aux[:, 2:3]  # 1.0
            ident = aux[:, 4:132]
            identf = singles.tile([P, P], f16)
            nc.vector.tensor_copy(identf[:], ident)

            def issue_loads(b):
                if b == 0:
                    # graded chunks: first matmul starts after first chunk
                    chunks = []
                    for ci in range(len(B0_BOUNDS) - 1):
                        lo, hi = B0_BOUNDS[ci], B0_BOUNDS[ci + 1]
                        t_ = xq_pool.tile(
                            [P, hi - lo, C], f16, tag=f"xq{ci}", name=f"xq{ci}"
                        )
                        nc.sync.dma_start(t_[:], xt_ap[b, :, lo:hi, :])
                        chunks.append((t_, lo, hi))
                else:
                    xta = xt_pool.tile([P, KH, C], f16, tag="xta", name="xta")
                    xtb = xt_pool.tile([P, KH, C], f16, tag="xtb", name="xtb")
                    nc.sync.dma_start(xta[:], xt_ap[b, :, :KH, :])
                    nc.sync.dma_start(xtb[:], xt_ap[b, :, KH:, :])
                    chunks = [(xta, 0, KH), (xtb, KH, KT)]
                xn = xn_pool.tile([P, 2, T], f8, tag="xn", name="xn")
                nc.sync.dma_start(xn[:], xn_ap[b])
                return chunks, xn

            def mm2_step(pb, pAt, prZ, pxn, m, t8, ot):
                """One DR matmul + evac of the (pb, m) output half, tile t8.
                Output DMA is issued after each quarter of the row block."""
                po = psum_o.tile([P, 512], f32)
                nc.tensor.matmul(
                    po[:],
                    lhsT=pAt[:, :, ts(m, P)],
                    rhs=pxn[:, :, ts(t8, 512)],
                    start=True,
                    stop=True,
                    perf_mode=DR,
                )
                if t8 % 2 == 0:
                    nc.vector.tensor_scalar_mul(
                        ot[:, ts(t8, 512)], po[:], prZ[:, m : m + 1]
                    )
                else:
                    nc.scalar.mul(
                        ot[:, ts(t8, 512)], po[:], prZ[:, m : m + 1]
                    )
                if t8 % 4 == 3:
                    nc.sync.dma_start(
                        o_ap[pb].rearrange("(m p) t -> p m t", p=P)[
                            :, m, (t8 - 3) * 512 : (t8 + 1) * 512
                        ],
                        ot[:, (t8 - 3) * 512 : (t8 + 1) * 512],
                    )

            def mm2_half(pb, pAt, prZ, pxn, m):
                ot = out_pool.tile([P, T], f16, tag="ot", name="ot")
                for t8 in range(TC):
                    mm2_step(pb, pAt, prZ, pxn, m, t8, ot)
                return ot

            tiles = {0: issue_loads(0)}
            pending = None

            for b in range(NB):
                xt, xn = tiles.pop(b)
                if b + 1 < NB:
                    tiles[b + 1] = issue_loads(b + 1)

                At = att_pool.tile([P, 2, C], f8)
                Zs = small.tile([P, 2], f32)
                Zb = small.tile([P, 2], f16)
                rZ = small.tile([P, 2], f32)
                prev = pending
                pending = (b, At, rZ, xn)
                deferred = []

                for m in range(2):
                    # previous batch's mm2 half m is interleaved into this
                    # half's mm1 k-loop: one DR matmul every 4 k-tiles keeps
                    # the PE dense while DVE/ACT drain the PSUM evacuations.
                    if prev is not None:
                        ot_prev = out_pool.tile([P, T], f16, tag="ot", name="ot")
                    pe = psum_e.tile([P, C], f32)
                    ci = 0
                    for k in range(KT):
                        while k >= xt[ci][2]:
                            ci += 1
                        src_t, lo, _ = xt[ci]
                        kk = k - lo
                        nc.tensor.matmul(
                            pe[:],
                            lhsT=src_t[:, kk, ts(m, P)],
                            rhs=src_t[:, kk, :],
                            start=(k == 0),
                            stop=(k == KT - 1),
                            skip_group_check=True,
                        )
                        if prev is not None and k % 4 == 3:
                            mm2_step(
                                prev[0], prev[1], prev[2], prev[3],
                                m, k // 4, ot_prev,
                            )
                    mu = small.tile([P, 1], f32)
                    nc.vector.tensor_reduce(
                        mu[:], pe[:], axis=mybir.AxisListType.X,
                        op=mybir.AluOpType.min,
                    )
                    Pm = small.tile([P, C], f16, tag=f"Pm{m}")
                    nc.scalar.activation(
                        Pm[:],
                        pe[:],
                        mybir.ActivationFunctionType.Exp,
                        bias=mu[:],
                        scale=-1.0,
                        accum_out=Zs[:, m : m + 1],
                    )
                    # Zb = Z/gamma (f16), rZ = gamma/Z (f32)
                    nc.vector.tensor_scalar_mul(
                        Zb[:, m : m + 1], Zs[:, m : m + 1], rgv
                    )
                    nc.vector.reciprocal(rZ[:, m : m + 1], Zb[:, m : m + 1])

                    def build(m, Pm, idx):
                        for k in range(2):
                            pt = psum_t.tile([P, P], f16)
                            nc.tensor.transpose(pt[:], Pm[:, ts(k, P)], idx[:])
                            nc.scalar.mul(At[:, k, ts(m, P)], pt[:], onev)

                    if prev is not None:
                        build(m, Pm, identf)
                    else:
                        # batch 0: defer builds past both softmax halves so
                        # the ACT queue runs [exp m0, exp m1] back-to-back
                        # and neither transpose stalls the PE on the chain
                        deferred.append((m, Pm))

                if prev is None:
                    for m_, Pm_ in deferred:
                        build(m_, Pm_, identf)

                if b == NB - 1:
                    mm2_half(b, At, rZ, xn, 0)
                    mm2_half(b, At, rZ, xn, 1)

    nc.compile()
    return nc


def _get_nc():
    if "nc" not in _CACHE:
        _CACHE["nc"] = _build_nc()
    return _CACHE["nc"]


def _make_aux(gamma_val):
    aux = np.zeros((P, 132), dtype=np.float32)
    aux[:, 0] = gamma_val
    aux[:, 1] = 1.0 / gamma_val if gamma_val != 0 else 0.0
    aux[:, 2] = 1.0
    aux[:, 4:132] = np.eye(P, dtype=np.float32)
    return aux


def kernel(x, gamma, _trace=False):
    import ml_dtypes

    import concourse.bass_utils as bass_utils

    x = np.ascontiguousarray(np.asarray(x, dtype=np.float32))
    gamma = np.asarray(gamma, dtype=np.float32).reshape(-1)

    nc = _get_nc()

    aux = _make_aux(gamma[0])
    x16 = x.astype(np.float16)
    in_maps = []
    for d in range(NCORES):
        xs16 = x16[d * NB : (d + 1) * NB]
        xt = np.ascontiguousarray(
            xs16.transpose(0, 2, 1).reshape(NB, KT, P, C).transpose(0, 2, 1, 3)
        )
        xs = x[d * NB : (d + 1) * NB]
        xn = np.ascontiguousarray(
            xs.reshape(NB, 2, P, T).transpose(0, 2, 1, 3)
        ).astype(ml_dtypes.float8_e4m3)
        in_maps.append({"xt": xt, "xn": xn, "aux": aux})

    res = bass_utils.run_bass_kernel_spmd(
        nc, in_maps, core_ids=list(range(NCORES)), trace=_trace
    )
    # device returns U = gamma * attn @ x (fp16); residual +x merged here
    out = np.concatenate([r["o"] for r in res.results], axis=0).astype(
        np.float32
    )
    out += x
    if _trace:
        _CACHE["last_results"] = res
    return out
